# revision 28
# baseline (speedup 1.0000x reference)
"""Trainium2 Bass kernel for the LNN Euler-Lagrange residual.

Math: for a ReLU MLP Lagrangian L(q, qdot) the JAX second-derivative term
d/dt(dL/dqdot) is identically zero (piecewise-linear network), so the
reference output reduces to -dL/dq:

    z1 = x @ W1 + b1          s1 = z1 > 0      a1 = relu(z1)
    z2 = a1 @ W2 + b2         s2 = z2 > 0
    pre1 = s2 @ W2T_eff       (W2T_eff[j,i] = w3[j] * W2[i, j])
    out  = (pre1 * s1) @ (-W1[:32,:].T)

Layout: feature-major (features on partitions, batch streams as matmul
free dim). Host pre-transposes the input shard to [64, B_core]. Two
batch groups are packed on the 128 partitions via host-built 128x128
block-diagonal / anti-diagonal stationary matrices, so every matmul
uses the full PE array with K=128.

GPSIMD cannot touch PSUM on TRN2, so every PSUM eviction runs on ACT or
DVE. To halve the per-op fixed costs the pipeline is pair-synchronous:
each stage processes a PAIR of 512-column blocks as one [128,1024]
eviction spanning two PSUM banks (two matmuls fill the halves). Four
PSUM pools (z1/z2/pre1/out) x 1 buf x 2 banks = all 8 banks. Per
pair-step: ACT does relu (f32r) + the sigmoid-saturated s2 mask (exact
{0,1} in fp16), DVE does the fused (a1>0)*pre1 multiply (fp16) + every
other step the [128,1024] output eviction. Stationaries S1/S2 ride one
f32r tensor (biases folded in), the value path (S3/S4/t1/out) is fp16.
The whole f32r input is DMA'd up front in graduated chunks; outputs
batch 2 pairs per store.
"""

import sys

sys.path.insert(0, "/opt/trn_rl_repo")

from contextlib import ExitStack

import numpy as np

B, D, H = 262144, 32, 64
NCORES = 8
BC = B // NCORES          # samples per core
G = BC // 2               # samples per group (2 groups on 128 partitions)
CHUNK = 512               # batch columns per block (pair = 2 blocks)

_CACHE = {}


def _round_f32r(a):
    """IEEE fp32 -> e8m11 (float32r): round mantissa to 11 bits (RNE)."""
    u = np.ascontiguousarray(a, np.float32).view(np.uint32)
    lsb = (u >> np.uint32(12)) & np.uint32(1)
    u2 = (u + np.uint32(0x7FF) + lsb) & np.uint32(0xFFFFF000)
    return u2.view(np.float32)


# input DMA chunk widths (columns); graduated so early pairs start early
XPLAN = [512, 512, 1024, 1024, 1024, 1024, 1024, 1024,
         1024, 1024, 1024, 1024, 1024, 2048, 2048]


def _build(bc, chunk, warm=30, bufs=None):
    import concourse.bass as bass
    import concourse.tile as tile
    from concourse import bacc, mybir

    f32 = mybir.dt.float32
    f32r = mybir.dt.float32r
    fp16 = mybir.dt.float16
    bf16 = mybir.dt.bfloat16
    Relu = mybir.ActivationFunctionType.Relu
    Copy = mybir.ActivationFunctionType.Copy
    Sigmoid = mybir.ActivationFunctionType.Sigmoid
    is_gt = mybir.AluOpType.is_gt
    mult = mybir.AluOpType.mult
    bypass = mybir.AluOpType.bypass

    g = bc // 2
    nb = g // chunk           # 32 blocks
    npair = nb // 2           # 16 pair-steps
    W2C = 2 * chunk           # 1024
    assert sum(XPLAN) == g
    # pair -> (chunk index, offset) for each half block
    blk_chunk = []
    ci, coff = 0, 0
    for b_ in range(nb):
        if coff >= XPLAN[ci]:
            ci += 1
            coff = 0
        blk_chunk.append((ci, coff))
        coff += chunk
    BUFS = {"a1": 5, "s2": 4, "t1": 4, "ot": 4}
    if bufs:
        BUFS.update(bufs)

    nc = bacc.Bacc("TRN2", target_bir_lowering=False, debug=False)

    xT = nc.dram_tensor("xT", [128, g], f32r, kind="ExternalInput").ap()
    # S1 | S2 | b1cat | -b2cat fused so one DMA carries the L1/L2 constants
    S12 = nc.dram_tensor("S12", [128, 258], f32r, kind="ExternalInput").ap()
    S34 = nc.dram_tensor("S34", [128, 192], fp16, kind="ExternalInput").ap()
    # outT rows (blocks of 32): A-even / B-even / A-odd / B-odd block outputs
    outT = nc.dram_tensor("outT", [128, g // 2], fp16, kind="ExternalOutput").ap()

    with tile.TileContext(nc) as tc, ExitStack() as ctx:
        wp = ctx.enter_context(tc.tile_pool(name="w", bufs=1))
        xs_p = ctx.enter_context(tc.tile_pool(name="xs", bufs=1))
        a1_p = ctx.enter_context(tc.tile_pool(name="a1", bufs=BUFS["a1"]))
        s2_p = ctx.enter_context(tc.tile_pool(name="s2", bufs=BUFS["s2"]))
        t1_p = ctx.enter_context(tc.tile_pool(name="t1", bufs=BUFS["t1"]))
        ot_p = ctx.enter_context(tc.tile_pool(name="ot", bufs=BUFS["ot"]))
        pz1 = ctx.enter_context(tc.tile_pool(name="pz1", bufs=1, space="PSUM"))
        pz2 = ctx.enter_context(tc.tile_pool(name="pz2", bufs=1, space="PSUM"))
        pp1 = ctx.enter_context(tc.tile_pool(name="pp1", bufs=1, space="PSUM"))
        pout = ctx.enter_context(tc.tile_pool(name="po", bufs=1, space="PSUM"))

        s12_t = wp.tile([128, 258], f32r, tag="s12")
        s34_t = wp.tile([128, 192], fp16, tag="s34")

        xs_tiles = []
        for k, w_ in enumerate(XPLAN):
            xs_tiles.append(xs_p.tile([128, w_], f32r, tag=f"xs{k}",
                                      name=f"xs{k}"))
        xoff = [0]
        for w_ in XPLAN[:-1]:
            xoff.append(xoff[-1] + w_)

        dum = wp.tile([128, 4], f32, tag="dum")
        wjunk = wp.tile([128, 128], bf16, tag="wjunk")
        nc.gpsimd.memset(dum[:], 0.0)
        nc.gpsimd.memset(wjunk[:], 0.0)

        # early DMA order tuned so pair k's data lands just before its L1s
        nc.sync.dma_start(out=s12_t[:], in_=S12)
        nc.scalar.dma_start(out=xs_tiles[0][:],
                            in_=xT[:, xoff[0]:xoff[0] + XPLAN[0]])
        # absorb the one-time LoadActFuncSet(s) (~1.3us each) off the
        # critical path: touch every ACT function the kernel uses
        nc.scalar.activation(out=dum[:], in_=dum[:], func=Relu, scale=1.0)
        nc.scalar.activation(out=dum[:], in_=dum[:], func=Sigmoid, scale=1.0)
        nc.scalar.activation(out=dum[:], in_=dum[:], func=Copy, scale=1.0)
        nc.sync.dma_start(out=xs_tiles[1][:],
                          in_=xT[:, xoff[1]:xoff[1] + XPLAN[1]])
        nc.scalar.dma_start(out=xs_tiles[2][:],
                            in_=xT[:, xoff[2]:xoff[2] + XPLAN[2]])
        nc.sync.dma_start(out=xs_tiles[3][:],
                          in_=xT[:, xoff[3]:xoff[3] + XPLAN[3]])
        nc.scalar.dma_start(out=s34_t[:], in_=S34)
        for k in range(4, len(XPLAN)):
            nc.sync.dma_start(out=xs_tiles[k][:],
                              in_=xT[:, xoff[k]:xoff[k] + XPLAN[k]])
        s1w = s12_t[:, 0:128]
        s2w = s12_t[:, 128:256]
        s3w = s34_t[:, 0:128]
        s4w = s34_t[:, 128:192]
        bia = s12_t[:, 256:258].bitcast(f32)

        # PE warm-up: junk bf16 matmuls (results never read) advance the
        # clock-gate ramp so real matmuls run at 2.4 GHz once data lands.
        warm_t = pz1.tile([128, W2C], f32, tag="pz1", name="warm")
        for _ in range(warm):
            nc.tensor.matmul(warm_t[:, 0:128], lhsT=wjunk[:], rhs=wjunk[:],
                             start=True, stop=True)

        a1s = {}
        s2ms = {}
        t1s = {}
        pouts = {}
        ots = {}

        def halves(P):
            ca, oa = blk_chunk[2 * P]
            cb, ob = blk_chunk[2 * P + 1]
            return (xs_tiles[ca][:, oa:oa + chunk],
                    xs_tiles[cb][:, ob:ob + chunk])

        # Pair-synchronous 5-stage pipeline; per pair-step P the PE stream is
        #   L1ab(P), L2ab(P-1), L3ab(P-3), L4ab(P-4)
        # with one [128,1024] eviction per engine slot:
        #   ACT: relu(P), sigmoid-mask(P-1); DVE: t1(P-3), out(2 pairs).
        # Fill/drain pairs run their evictions per-half ([128,512]) so the
        # latency chains at the ends stay block-granular.
        SPLIT = {0, 1, npair - 4, npair - 3, npair - 2, npair - 1}

        def ev_halves(P):
            if P in SPLIT:
                return [(0, chunk), (chunk, W2C)]
            return [(0, W2C)]

        for P in range(npair + 4):
            if P < npair:
                xa, xb = halves(P)
                z1p = pz1.tile([128, W2C], f32, tag="pz1", name="z1p")
                a1 = a1_p.tile([128, W2C], f32r, tag="a1", name="a1")
                nc.tensor.matmul(z1p[:, 0:chunk], lhsT=s1w, rhs=xa,
                                 start=True, stop=True)
                if P in SPLIT:
                    nc.scalar.activation(out=a1[:, 0:chunk],
                                         in_=z1p[:, 0:chunk], func=Relu,
                                         bias=bia[:, 0:1], scale=1.0)
                nc.tensor.matmul(z1p[:, chunk:W2C], lhsT=s1w, rhs=xb,
                                 start=True, stop=True)
                if P in SPLIT:
                    nc.scalar.activation(out=a1[:, chunk:W2C],
                                         in_=z1p[:, chunk:W2C], func=Relu,
                                         bias=bia[:, 0:1], scale=1.0)
                else:
                    nc.scalar.activation(out=a1[:], in_=z1p[:], func=Relu,
                                         bias=bia[:, 0:1], scale=1.0)
                a1s[P] = a1

            if 0 <= P - 1 < npair:
                i = P - 1
                a1 = a1s[i]
                z2p = pz2.tile([128, W2C], f32, tag="pz2", name="z2p")
                # s2 = (z2 > -b2) as exact {0,1} fp16: sigmoid(1e30*(z2+b2))
                # saturates; keeps the mask op on ACT where it is cheapest
                s2m = s2_p.tile([128, W2C], fp16, tag="s2", name="s2m")
                nc.tensor.matmul(z2p[:, 0:chunk], lhsT=s2w,
                                 rhs=a1[:, 0:chunk], start=True, stop=True)
                if i in SPLIT:
                    nc.scalar.activation(out=s2m[:, 0:chunk],
                                         in_=z2p[:, 0:chunk], func=Sigmoid,
                                         bias=bia[:, 1:2], scale=1e30)
                nc.tensor.matmul(z2p[:, chunk:W2C], lhsT=s2w,
                                 rhs=a1[:, chunk:W2C], start=True, stop=True)
                if i in SPLIT:
                    nc.scalar.activation(out=s2m[:, chunk:W2C],
                                         in_=z2p[:, chunk:W2C], func=Sigmoid,
                                         bias=bia[:, 1:2], scale=1e30)
                else:
                    nc.scalar.activation(out=s2m[:], in_=z2p[:], func=Sigmoid,
                                         bias=bia[:, 1:2], scale=1e30)
                s2ms[i] = s2m

            if 0 <= P - 3 < npair:
                i = P - 3
                s2m = s2ms.pop(i)
                a1 = a1s.pop(i)
                p1p = pp1.tile([128, W2C], f32, tag="pp1", name="p1p")
                t1 = t1_p.tile([128, W2C], fp16, tag="t1", name="t1")
                nc.tensor.matmul(p1p[:, 0:chunk], lhsT=s3w,
                                 rhs=s2m[:, 0:chunk], start=True, stop=True)
                if i in SPLIT:
                    nc.vector.scalar_tensor_tensor(
                        out=t1[:, 0:chunk],
                        in0=a1[:, 0:chunk].bitcast(f32), scalar=0.0,
                        in1=p1p[:, 0:chunk], op0=is_gt, op1=mult)
                nc.tensor.matmul(p1p[:, chunk:W2C], lhsT=s3w,
                                 rhs=s2m[:, chunk:W2C], start=True, stop=True)
                if i in SPLIT:
                    nc.vector.scalar_tensor_tensor(
                        out=t1[:, chunk:W2C],
                        in0=a1[:, chunk:W2C].bitcast(f32), scalar=0.0,
                        in1=p1p[:, chunk:W2C], op0=is_gt, op1=mult)
                else:
                    nc.vector.scalar_tensor_tensor(
                        out=t1[:], in0=a1[:].bitcast(f32),
                        scalar=0.0, in1=p1p[:], op0=is_gt, op1=mult)
                t1s[i] = t1

            if 0 <= P - 4 < npair:
                i = P - 4
                par = i % 2
                Q = i // 2
                if par == 0:
                    pouts[Q] = pout.tile([128, W2C], f32, tag="po",
                                         name="outp")
                op_ = pouts[Q]
                t1 = t1s.pop(i)
                base = par * chunk
                nc.tensor.matmul(op_[0:64, base:base + chunk], lhsT=s4w,
                                 rhs=t1[:, 0:chunk], start=True, stop=True)
                nc.tensor.matmul(op_[64:128, base:base + chunk], lhsT=s4w,
                                 rhs=t1[:, chunk:W2C], start=True, stop=True)
                if par == 1:
                    ot = ot_p.tile([128, W2C], fp16, tag="ot", name="ot")
                    last = i == npair - 1
                    if last:
                        # drain: per-half on the idle ACT ring with an
                        # immediate store per half to shorten the tail chain
                        nc.scalar.activation(out=ot[:, 0:chunk],
                                             in_=op_[:, 0:chunk], func=Copy)
                        nc.sync.dma_start(
                            out=outT[:, Q * W2C:Q * W2C + chunk],
                            in_=ot[:, 0:chunk])
                        nc.scalar.activation(out=ot[:, chunk:W2C],
                                             in_=op_[:, chunk:W2C],
                                             func=Copy)
                        nc.sync.dma_start(
                            out=outT[:, Q * W2C + chunk:(Q + 1) * W2C],
                            in_=ot[:, chunk:W2C])
                    else:
                        nc.vector.tensor_scalar(out=ot[:], in0=op_[:],
                                                scalar1=0.0, scalar2=None,
                                                op0=bypass)
                        nc.sync.dma_start(
                            out=outT[:, Q * W2C:(Q + 1) * W2C], in_=ot[:, :])
                    del pouts[Q]

    nc.compile()
    return nc


def _get_nc(bc=BC, chunk=CHUNK, **kw):
    key = (bc, chunk, str(kw))
    if key not in _CACHE:
        _CACHE[key] = _build(bc, chunk, **kw)
    return _CACHE[key]


def _host_prep(W1, b1, W2, b2, W3, b3):
    w3 = np.asarray(W3)[:, 0].astype(np.float32)
    W1 = np.asarray(W1, np.float32)
    W2 = np.asarray(W2, np.float32)
    b1 = np.asarray(b1, np.float32)
    b2 = np.asarray(b2, np.float32)

    S12 = np.zeros((128, 258), np.float32)
    S12[:64, 0:64] = W1
    S12[64:, 64:128] = W1
    S12[:64, 192:256] = W2
    S12[64:, 128:192] = W2
    S12[:, 256] = np.concatenate([b1, b1])
    S12[:, 257] = -np.concatenate([b2, b2])
    S3s = (W2 * w3[None, :]).T  # [j, i] = w3[j] * W2[i, j]
    S34 = np.zeros((128, 192), np.float32)
    S34[64:, 0:64] = S3s    # A: s2 at p64:128 -> pre1 at p0:64
    S34[:64, 64:128] = S3s  # B: s2 at p0:64   -> pre1 at p64:128
    S4s = -(W1[:32, :].T)   # [64, 32]
    S34[:64, 128:160] = S4s  # A: t1 p0:64   -> out p0:32 (+64 odd blocks)
    S34[64:, 160:192] = S4s  # B: t1 p64:128 -> out p32:64 (+64 odd blocks)
    return {
        "S12": _round_f32r(S12),
        "S34": S34.astype(np.float16),
    }


def kernel(inputs, W1, b1, W2, b2, W3, b3):
    from concourse.bass_utils import run_bass_kernel_spmd

    x = np.ascontiguousarray(np.asarray(inputs, np.float32))
    consts = _host_prep(W1, b1, W2, b2, W3, b3)

    in_maps = []
    for k in range(NCORES):
        xc = x[k * BC:(k + 1) * BC]          # [BC, 64]
        # rows p = grp*64+f: group A samples [0,G) then group B [G,2G)
        xTk = _round_f32r(np.ascontiguousarray(
            np.concatenate([xc[:G].T, xc[G:].T], axis=0)))
        in_maps.append({"xT": xTk, **consts})

    nc = _get_nc()
    res = run_bass_kernel_spmd(nc, in_maps, core_ids=list(range(NCORES)),
                               trace=False)
    outs = []
    for k in range(NCORES):
        oT = np.asarray(res.results[k]["outT"]).astype(np.float32)
        a = np.empty((G, 32), np.float32)
        b = np.empty((G, 32), np.float32)
        for p in range(G // (2 * CHUNK)):
            blk = oT[:, p * CHUNK:(p + 1) * CHUNK]
            se, so = 2 * p * CHUNK, (2 * p + 1) * CHUNK
            # rows: 4 groups of 32 = A-even / B-even / A-odd / B-odd
            a[se:se + CHUNK] = blk[0:32].T
            b[se:se + CHUNK] = blk[32:64].T
            a[so:so + CHUNK] = blk[64:96].T
            b[so:so + CHUNK] = blk[96:128].T
        outs.append(a)
        outs.append(b)
    out = np.concatenate(outs, axis=0).astype(np.float32)
    kernel._last_result = res
    return out
